# revision 33
# baseline (speedup 1.0000x reference)
"""LoG (GaussianBlur 3x3 then Laplacian 9x9, reflect-101) as a Bass/Trainium kernel.

Math: both depthwise convs are separable and symmetric, and reflect-101
padding commutes with symmetric-kernel convolution, so the whole pipeline
composes into a single separable 11x11 filter applied to the reflect-101
extension of x:

    out = clip( (A (x) B + B (x) A) * x~  + 1, 0, 255 )

with A = gauss3 conv SMOOTH_9 (11 taps), B = gauss3 conv D2_9 (11 taps).

The run is DMA-fabric bound: the 16 device DMA engines sustain ~360 GB/s
shared by all 8 cores (~45 GB/s/core measured), so bytes moved dominate.
Input is therefore shipped as 2 bytes/elem: uint16 quantization with
noise-SHAPED dithering. The quantization error is spectrally shaped (33
feedback taps over the 3 previously-quantized rows, fully vectorized on
host) into the bands where the composed LoG filter has near-zero
response; measured end-to-end error of the u16 path is ~1.6 absolute
(vs 5.8 for naive u16 and a 5.1 budget incl. margin for other sources).
Output is uint8 (saturating engine cast). Bands ship as an fp16 hi/lo
pair; the fp32 stage-2 band is reconstructed on device (saves DMA).

Per core (batch 4 of the 32 images):
  unpack:  x_hi = f16(u16 * s) on Vector; x_lo = (u16 * s) - x_hi on
      GpSimd (scalar_tensor_tensor), both one pass per x-row chunk.
  stage 1 (y-conv, transposed output for free), fp16 matmuls (1 cyc/row
      vs fp32's 4): psum[x_chunk, 206] accumulates three matmuls
      x_hi^T@[A|B]_hi + x_lo^T@[A|B]_hi + x_hi^T@[A|B]_lo (band split
      hi/lo keeps the taps exact to 2^-21; dropped lo*lo is negligible).
      Two y-chunks share one PSUM bank, drained by a single 4D-AP copy
      into uv tiles.
  stage 2 (x-conv, back to original orientation), fp32 matmuls (u,v are
      ~6.5e4-scale; fp16 there would break the error budget): all 5
      x-chunks of one 128-row block accumulate into ONE PSUM bank (10
      matmuls), then a single Relu(psum + 1) activation per bank writes
      the uint8 output tile (saturates at 255, so the clip is free).

Big DMAs round-robin over the 3 DMA-capable queues (sync/scalar/gpsimd);
stage-1 drains go mostly to Vector. Reflect-101 boundaries are folded
into the per-chunk band matrices. x-chunks carry a +-5 y-halo so stage-2
lhsT reads stay partition-aligned.
"""

import numpy as np

N_CORES = 8
BATCH = 32
IMG_PER_CORE = BATCH // N_CORES
H = W = 512
C = 3
RAD = 5  # half width of composed 11-tap filter

QS = np.float32(255.0 / 65535.0)  # u16 quantization step


def make_chunks(n):
    step = 103  # 103*5 >= 512, in-size <= 113 <= 128
    bounds = list(range(0, n, step)) + [n]
    out = []
    for s, e in zip(bounds[:-1], bounds[1:]):
        lo, hi = max(s - RAD, 0), min(e + RAD, n)
        out.append((s, e, lo, hi))
    return out


CHUNKS = make_chunks(H)


def make_taps():
    g = np.exp(-((np.arange(3) - 1.0) ** 2) / 2.0)
    g = g / g.sum()
    S = np.array([1, 8, 28, 56, 70, 56, 28, 8, 1], dtype=np.float64)
    D2 = np.array([1, 4, 4, -4, -10, -4, 4, 4, 1], dtype=np.float64)
    return np.convolve(g, S), np.convolve(g, D2)


def make_bands(n):
    """Per chunk: [K, 2*Ni] = [BandA | BandB], reflect-101 folded in."""
    A, B = make_taps()
    bands = []
    for s, e, lo, hi in make_chunks(n):
        ni = e - s
        w = np.zeros((hi - lo, 2 * ni), np.float64)
        for j in range(ni):
            y = s + j
            for t in range(-RAD, RAD + 1):
                src = y + t
                if src < 0:
                    src = -src
                elif src > n - 1:
                    src = 2 * (n - 1) - src
                w[src - lo, j] += A[t + RAD]
                w[src - lo, ni + j] += B[t + RAD]
        bands.append(w)
    return bands


def _pairs(n):
    ps, i = [], 0
    while i < n:
        ps.append(tuple(range(i, min(i + 2, n))))
        i += 2
    return ps


def _shaping_taps():
    """LS-optimal noise-shaping feedback taps (3 previous rows, |dx|<=5),
    minimizing quantization-noise power through the composed LoG filter."""
    A, B = make_taps()
    lap2d = np.outer(A, B) + np.outer(B, A)
    N = 128
    f = np.fft.fftfreq(N) * 2 * np.pi
    FY, FX = np.meshgrid(f, f, indexing="ij")
    L = np.zeros((N, N), complex)
    for dy in range(11):
        for dx in range(11):
            L += lap2d[dy, dx] * np.exp(-1j * (FY * (dy - 5) + FX * (dx - 5)))
    P = np.abs(L) ** 2
    locs = [(dy, dx) for dy in range(1, 4) for dx in range(-5, 6)]
    E = [np.exp(-1j * (FY * dy + FX * dx)) for (dy, dx) in locs]
    n = len(E)
    M = np.zeros((n, n))
    b = np.zeros(n)
    for i in range(n):
        b[i] = -(np.real(E[i]) * P).mean()
        for j in range(i, n):
            M[i, j] = M[j, i] = (np.real(E[i] * np.conj(E[j])) * P).mean()
    t = np.linalg.solve(M, b)
    return locs, -t  # feedback taps: NTF = 1 - sum(fb z^-k) = 1 + sum(t z^-k)


_SHAPE_LOCS, _SHAPE_TAPS = _shaping_taps()


def encode_x(x):
    """Noise-shaped u16 quantization of x [N,H,W,C], vectorized over
    everything but the row index (error feedback over 3 previous rows)."""
    x = np.ascontiguousarray(x, dtype=np.float32)
    n, h, w, c = x.shape
    qs = np.float32(QS)
    err = np.zeros((4, n, w, c), np.float32)
    out = np.empty((n, h, w, c), np.uint16)
    for y in range(h):
        fb = np.zeros((n, w, c), np.float32)
        for (dy, dx), t in zip(_SHAPE_LOCS, _SHAPE_TAPS):
            e = err[dy]
            if dx == 0:
                fb += np.float32(t) * e
            elif dx > 0:
                fb[:, dx:, :] += np.float32(t) * e[:, :-dx, :]
            else:
                fb[:, :dx, :] += np.float32(t) * e[:, -dx:, :]
        target = x[:, y] - fb
        q = np.clip(np.rint(target / qs), 0, 65535)
        out[:, y] = q.astype(np.uint16)
        err = np.roll(err, 1, axis=0)
        err[1] = q.astype(np.float32) * qs - target
        err[0] = 0.0
    return out


def build_bass(n_imgs=IMG_PER_CORE, h=H, w=W, c=C):
    import concourse.bacc as bacc
    import concourse.mybir as mybir
    import concourse.tile as tile

    f32 = mybir.dt.float32
    f16 = mybir.dt.float16
    u8 = mybir.dt.uint8
    u16 = mybir.dt.uint16
    relu = mybir.ActivationFunctionType.Relu
    chunks = make_chunks(h)
    assert w == h, "chunking shared across axes"
    jpairs = _pairs(len(chunks))

    nc = bacc.Bacc("TRN2", target_bir_lowering=False, debug=False)
    xq_d = nc.dram_tensor("x_q", [n_imgs, h, w, c], u16, kind="ExternalInput")
    out_d = nc.dram_tensor("out", [n_imgs, h, w, c], u8, kind="ExternalOutput")
    # all chunk bands packed into one tensor pair: 412-byte-per-line
    # transfers run at ~2 GB/s (sub-packet descriptors); one 4KB-per-line
    # transfer streams at full rate
    nck = len(chunks)
    bandh_d = nc.dram_tensor("bandh", [128, 206 * nck], f16, kind="ExternalInput")
    bandl_d = nc.dram_tensor("bandl", [128, 206 * nck], f16, kind="ExternalInput")

    n_yblk = h // 128

    with tile.TileContext(nc) as tc:
        with (
            tc.tile_pool(name="const", bufs=1) as cpool,
            tc.tile_pool(name="xin", bufs=2) as xpool,
            tc.tile_pool(name="uv", bufs=1) as uvpool,
            tc.tile_pool(name="outp", bufs=2) as opool,
            tc.tile_pool(name="ps", bufs=3, space="PSUM") as pspool,
            tc.tile_pool(name="pso", bufs=3, space="PSUM") as psopool,
        ):
            # spread big DMAs over the 3 DMA-capable engine queues (a single
            # queue serializes and the 16-engine fabric is the bottleneck)
            dma_engines = [nc.sync, nc.scalar, nc.gpsimd]
            dma_rr = [0]

            def dma(dst, src):
                eng = dma_engines[dma_rr[0] % len(dma_engines)]
                dma_rr[0] += 1
                eng.dma_start(dst, src)

            # band constants in 3 per-chunk-pair piece tiles so each piece
            # can load just-in-time, interleaved with image-0's x chunks
            # (a monolithic band load ahead of the x loads starves the ramp)
            pieces = [(0, 412), (412, 824), (824, 206 * nck)]
            bandh_p, bandl_p, band32_p = [], [], []
            for k, (c0, c1) in enumerate(pieces):
                bh = cpool.tile([128, c1 - c0], f16, name=f"bandh_p{k}")
                bl = cpool.tile([128, c1 - c0], f16, name=f"bandl_p{k}")
                b32 = cpool.tile([128, c1 - c0], f32, name=f"band32_p{k}")
                bandh_p.append(bh)
                bandl_p.append(bl)
                band32_p.append(b32)

            def load_band_piece(k):
                c0, c1 = pieces[k]
                rows = [0, 43, 86, 128]
                for q in range(3):
                    a, bnd = rows[q], rows[q + 1]
                    dma_engines[q].dma_start(
                        bandh_p[k][a:bnd, :], bandh_d.ap()[a:bnd, c0:c1]
                    )
                    dma_engines[q].dma_start(
                        bandl_p[k][a:bnd, :], bandl_d.ap()[a:bnd, c0:c1]
                    )
                nc.vector.tensor_tensor(
                    band32_p[k][:], bandh_p[k][:], bandl_p[k][:],
                    mybir.AluOpType.add,
                )

            bandh, bandl, bandB, bandA = [], [], [], []
            for i, (s, e, lo, hi) in enumerate(chunks):
                rows, ni = hi - lo, e - s
                k, off = i // 2, (i % 2) * 206
                bandh.append(bandh_p[k][0:rows, off : off + 2 * ni])
                bandl.append(bandl_p[k][0:rows, off : off + 2 * ni])
                bandA.append(band32_p[k][0:rows, off : off + ni])
                bandB.append(band32_p[k][0:rows, off + ni : off + 2 * ni])

            # stage-1 psum drains: GpSimd cannot access PSUM; Vector takes
            # most (it cannot issue DMAs, so it has the most slack)
            drain_engines = [
                nc.vector.tensor_copy,
                nc.vector.tensor_copy,
                nc.scalar.copy,
                nc.vector.tensor_copy,
                nc.scalar.copy,
            ]

            def issue_dmas(n):
                """Issue image n's chunk loads, 3-way queue-split. Called one
                iteration ahead so the doorbells land in the Scalar/GpSimd
                instruction streams BEFORE image n-1's drains/activations
                (otherwise those queues idle, then burst at the boundary)."""
                xus = []
                for j, (s, e, lo, hi) in enumerate(chunks):
                    # just-in-time band piece loads, interleaved with image
                    # 0's chunks in queue order
                    if n == 0 and j % 2 == 0:
                        load_band_piece(j // 2)
                    xu = xpool.tile([hi - lo, w, c], u16, tag=f"xu{j}", name=f"xu{j}_{n}")
                    rows = hi - lo
                    cuts = [0, 32, 64, rows]
                    for q in range(3):
                        a, bnd = cuts[q], cuts[q + 1]
                        dma_engines[q].dma_start(
                            xu[a:bnd, :, :], xq_d.ap()[n, lo + a : lo + bnd, :, :]
                        )
                    xus.append(xu)
                return xus

            def issue_unpacks(xus, n):
                """Unpack image n's chunks piece-by-piece on Vector. Kept at
                the consuming iteration (data already landed) so Vector never
                blocks on in-flight DMAs ahead of the psum drains.
                x_hi = f16(u16*s); x_lo = u16*s - x_hi. 32-aligned piece
                starts: vector ops require base partitions at multiples
                of 32."""
                xhis, xlos = [], []
                for j, (s, e, lo, hi) in enumerate(chunks):
                    xu = xus[j]
                    xh = xpool.tile([hi - lo, w, c], f16, tag=f"xh{j}", name=f"xh{j}_{n}")
                    xl = xpool.tile([hi - lo, w, c], f16, tag=f"xl{j}", name=f"xl{j}_{n}")
                    rows = hi - lo
                    cuts = [0, 32, 64, rows]
                    for q in range(3):
                        a, bnd = cuts[q], cuts[q + 1]
                        nc.vector.tensor_scalar(
                            xh[a:bnd, :, :], xu[a:bnd, :, :],
                            float(QS), None, mybir.AluOpType.mult,
                        )
                        nc.vector.scalar_tensor_tensor(
                            xl[a:bnd, :, :], xu[a:bnd, :, :], float(QS),
                            xh[a:bnd, :, :],
                            mybir.AluOpType.mult, mybir.AluOpType.subtract,
                        )
                    xhis.append(xh)
                    xlos.append(xl)
                return xhis, xlos

            pending_xu = issue_dmas(0)
            for n in range(n_imgs):
                xu_n = pending_xu
                if n + 1 < n_imgs:
                    pending_xu = issue_dmas(n + 1)
                xhis, xlos = issue_unpacks(xu_n, n)
                outs = []
                for b in range(n_yblk):
                    ot = opool.tile([128, w, c], u8, tag=f"o{b}", name=f"o{b}_{n}")
                    outs.append(ot)
                # uv tiles: plane 0 = u, plane 1 = v (transposed: x on
                # partitions); all 3 channels alive at once so the ramp
                # (image-0 chunks arriving at fabric rate) can feed the PE
                # with 3 channels' worth of stage-1 work per chunk pair
                uvts = [
                    [
                        uvpool.tile(
                            [hi - lo, 2, h], f32, tag=f"uv{ci}_{i}",
                            name=f"uv{ci}_{i}_{n}",
                        )
                        for i, (s, e, lo, hi) in enumerate(chunks)
                    ]
                    for ci in range(c)
                ]
                # stage 1: y-conv, transposed outputs; 3 fp16 matmuls per
                # (i,j); chunk-pair outer, channel inner
                for jp in jpairs:
                    nj = chunks[jp[0]][1] - chunks[jp[0]][0]
                    sj0 = chunks[jp[0]][0]
                    seg = 2 * nj
                    for ci in range(c):
                        for i, (si, ei, loi, hii) in enumerate(chunks):
                            mi = hii - loi
                            ps = pspool.tile([mi, 512], f32, tag="ps")
                            for t, j in enumerate(jp):
                                win = ps[:, t * seg : (t + 1) * seg]
                                hi_lhs = xhis[j][:, loi:hii, ci]
                                lo_lhs = xlos[j][:, loi:hii, ci]
                                nc.tensor.matmul(
                                    win, hi_lhs, bandh[j],
                                    start=True, stop=False,
                                )
                                nc.tensor.matmul(
                                    win, lo_lhs, bandh[j],
                                    start=False, stop=False,
                                )
                                nc.tensor.matmul(
                                    win, hi_lhs, bandl[j],
                                    start=False, stop=True,
                                )
                            # drain all segments with one 4D-AP copy
                            src = (
                                ps[:, 0 : len(jp) * seg]
                                .rearrange("m (js x) -> m js x", js=len(jp))
                                .rearrange("m js (uv x) -> m uv js x", uv=2)
                            )
                            dst = uvts[ci][i][
                                :, :, sj0 : sj0 + len(jp) * nj
                            ].rearrange("m uv (js x) -> m uv js x", js=len(jp))
                            drain_engines[i % 5](dst, src)
                # stage 2: x-conv; all 5 chunks accumulate into one PSUM
                # bank per 128-row block, single Relu(+1)->uint8 drain
                for ci in range(c):
                    for b in range(n_yblk):
                        pso = psopool.tile([128, 512], f32, tag="pso")
                        col = 0
                        for i, (s_, e_, lo_, hi_) in enumerate(chunks):
                            ni = e_ - s_
                            oslice = pso[:, col : col + ni]
                            nc.tensor.matmul(
                                oslice,
                                uvts[ci][i][:, 0, b * 128 : (b + 1) * 128],
                                bandB[i],
                                start=True,
                                stop=False,
                            )
                            nc.tensor.matmul(
                                oslice,
                                uvts[ci][i][:, 1, b * 128 : (b + 1) * 128],
                                bandA[i],
                                start=False,
                                stop=True,
                            )
                            col += ni
                        # Relu(psum + 1) -> uint8 saturates at 255: full clip
                        nc.scalar.activation(
                            outs[b][:, :, ci], pso[:, 0:col], relu, bias=1.0
                        )
                for b in range(n_yblk):
                    dma(out_d.ap()[n, b * 128 : (b + 1) * 128, :, :], outs[b][:])

    nc.compile()
    return nc


_CACHE = {}


def _get_nc():
    if "nc" not in _CACHE:
        _CACHE["nc"] = build_bass()
    return _CACHE["nc"]


def kernel(x: np.ndarray) -> np.ndarray:
    from concourse import bass_utils

    nc = _get_nc()
    bands64 = make_bands(H)
    nck = len(bands64)
    packh = np.zeros((128, 206 * nck), np.float16)
    packl = np.zeros((128, 206 * nck), np.float16)
    for i, b64 in enumerate(bands64):
        rows, seg = b64.shape
        bh = b64.astype(np.float16)
        bl = (b64 - bh.astype(np.float64)).astype(np.float16)
        packh[0:rows, 206 * i : 206 * i + seg] = bh
        packl[0:rows, 206 * i : 206 * i + seg] = bl
    const_map = {"bandh": packh, "bandl": packl}
    x_q = encode_x(x)
    in_maps = [
        {
            "x_q": x_q[k * IMG_PER_CORE : (k + 1) * IMG_PER_CORE],
            **const_map,
        }
        for k in range(N_CORES)
    ]
    res = bass_utils.run_bass_kernel_spmd(nc, in_maps, core_ids=list(range(N_CORES)))
    _CACHE["last_result"] = res
    out = np.concatenate([r["out"] for r in res.results], axis=0)
    return out.astype(np.float32)


# revision 35
# speedup vs baseline: 1.0338x; 1.0338x over previous
"""LoG (GaussianBlur 3x3 then Laplacian 9x9, reflect-101) as a Bass/Trainium kernel.

Math: both depthwise convs are separable and symmetric, and reflect-101
padding commutes with symmetric-kernel convolution, so the whole pipeline
composes into a single separable 11x11 filter applied to the reflect-101
extension of x:

    out = clip( (A (x) B + B (x) A) * x~  + 1, 0, 255 )

with A = gauss3 conv SMOOTH_9 (11 taps), B = gauss3 conv D2_9 (11 taps).

The run is DMA-fabric bound: the 16 device DMA engines sustain ~360 GB/s
shared by all 8 cores (~45 GB/s/core measured), so bytes moved dominate.
Input is therefore shipped as 2 bytes/elem: uint16 quantization with
noise-SHAPED dithering. The quantization error is spectrally shaped (33
feedback taps over the 3 previously-quantized rows, fully vectorized on
host) into the bands where the composed LoG filter has near-zero
response; measured end-to-end error of the u16 path is ~1.6 absolute
(vs 5.8 for naive u16 and a 5.1 budget incl. margin for other sources).
Output is uint8 (saturating engine cast). Bands ship as an fp16 hi/lo
pair; the fp32 stage-2 band is reconstructed on device (saves DMA).

Per core (batch 4 of the 32 images):
  unpack:  x_hi = f16(u16 * s) on Vector; x_lo = (u16 * s) - x_hi on
      GpSimd (scalar_tensor_tensor), both one pass per x-row chunk.
  stage 1 (y-conv, transposed output for free), fp16 matmuls (1 cyc/row
      vs fp32's 4): psum[x_chunk, 206] accumulates three matmuls
      x_hi^T@[A|B]_hi + x_lo^T@[A|B]_hi + x_hi^T@[A|B]_lo (band split
      hi/lo keeps the taps exact to 2^-21; dropped lo*lo is negligible).
      Two y-chunks share one PSUM bank, drained by a single 4D-AP copy
      into uv tiles.
  stage 2 (x-conv, back to original orientation), fp32 matmuls (u,v are
      ~6.5e4-scale; fp16 there would break the error budget): all 5
      x-chunks of one 128-row block accumulate into ONE PSUM bank (10
      matmuls), then a single Relu(psum + 1) activation per bank writes
      the uint8 output tile (saturates at 255, so the clip is free).

Big DMAs round-robin over the 3 DMA-capable queues (sync/scalar/gpsimd);
stage-1 drains go mostly to Vector. Reflect-101 boundaries are folded
into the per-chunk band matrices. x-chunks carry a +-5 y-halo so stage-2
lhsT reads stay partition-aligned.
"""

import numpy as np

N_CORES = 8
BATCH = 32
IMG_PER_CORE = BATCH // N_CORES
H = W = 512
C = 3
RAD = 5  # half width of composed 11-tap filter

QS = np.float32(255.0 / 65535.0)  # u16 quantization step


def make_chunks(n):
    step = 103  # 103*5 >= 512, in-size <= 113 <= 128
    bounds = list(range(0, n, step)) + [n]
    out = []
    for s, e in zip(bounds[:-1], bounds[1:]):
        lo, hi = max(s - RAD, 0), min(e + RAD, n)
        out.append((s, e, lo, hi))
    return out


CHUNKS = make_chunks(H)


def make_taps():
    g = np.exp(-((np.arange(3) - 1.0) ** 2) / 2.0)
    g = g / g.sum()
    S = np.array([1, 8, 28, 56, 70, 56, 28, 8, 1], dtype=np.float64)
    D2 = np.array([1, 4, 4, -4, -10, -4, 4, 4, 1], dtype=np.float64)
    return np.convolve(g, S), np.convolve(g, D2)


def make_bands(n):
    """Per chunk: [K, 2*Ni] = [BandA | BandB], reflect-101 folded in."""
    A, B = make_taps()
    bands = []
    for s, e, lo, hi in make_chunks(n):
        ni = e - s
        w = np.zeros((hi - lo, 2 * ni), np.float64)
        for j in range(ni):
            y = s + j
            for t in range(-RAD, RAD + 1):
                src = y + t
                if src < 0:
                    src = -src
                elif src > n - 1:
                    src = 2 * (n - 1) - src
                w[src - lo, j] += A[t + RAD]
                w[src - lo, ni + j] += B[t + RAD]
        bands.append(w)
    return bands


def _pairs(n):
    ps, i = [], 0
    while i < n:
        ps.append(tuple(range(i, min(i + 2, n))))
        i += 2
    return ps


def _shaping_taps():
    """LS-optimal noise-shaping feedback taps (3 previous rows, |dx|<=5),
    minimizing quantization-noise power through the composed LoG filter."""
    A, B = make_taps()
    lap2d = np.outer(A, B) + np.outer(B, A)
    N = 128
    f = np.fft.fftfreq(N) * 2 * np.pi
    FY, FX = np.meshgrid(f, f, indexing="ij")
    L = np.zeros((N, N), complex)
    for dy in range(11):
        for dx in range(11):
            L += lap2d[dy, dx] * np.exp(-1j * (FY * (dy - 5) + FX * (dx - 5)))
    P = np.abs(L) ** 2
    locs = [(dy, dx) for dy in range(1, 4) for dx in range(-5, 6)]
    E = [np.exp(-1j * (FY * dy + FX * dx)) for (dy, dx) in locs]
    n = len(E)
    M = np.zeros((n, n))
    b = np.zeros(n)
    for i in range(n):
        b[i] = -(np.real(E[i]) * P).mean()
        for j in range(i, n):
            M[i, j] = M[j, i] = (np.real(E[i] * np.conj(E[j])) * P).mean()
    t = np.linalg.solve(M, b)
    return locs, -t  # feedback taps: NTF = 1 - sum(fb z^-k) = 1 + sum(t z^-k)


_SHAPE_LOCS, _SHAPE_TAPS = _shaping_taps()


def encode_x(x):
    """Noise-shaped u16 quantization of x [N,H,W,C], vectorized over
    everything but the row index (error feedback over 3 previous rows)."""
    x = np.ascontiguousarray(x, dtype=np.float32)
    n, h, w, c = x.shape
    qs = np.float32(QS)
    err = np.zeros((4, n, w, c), np.float32)
    out = np.empty((n, h, w, c), np.uint16)
    for y in range(h):
        fb = np.zeros((n, w, c), np.float32)
        for (dy, dx), t in zip(_SHAPE_LOCS, _SHAPE_TAPS):
            e = err[dy]
            if dx == 0:
                fb += np.float32(t) * e
            elif dx > 0:
                fb[:, dx:, :] += np.float32(t) * e[:, :-dx, :]
            else:
                fb[:, :dx, :] += np.float32(t) * e[:, -dx:, :]
        target = x[:, y] - fb
        q = np.clip(np.rint(target / qs), 0, 65535)
        out[:, y] = q.astype(np.uint16)
        err = np.roll(err, 1, axis=0)
        err[1] = q.astype(np.float32) * qs - target
        err[0] = 0.0
    return out


def build_bass(n_imgs=IMG_PER_CORE, h=H, w=W, c=C):
    import concourse.bacc as bacc
    import concourse.mybir as mybir
    import concourse.tile as tile

    f32 = mybir.dt.float32
    f16 = mybir.dt.float16
    u8 = mybir.dt.uint8
    u16 = mybir.dt.uint16
    relu = mybir.ActivationFunctionType.Relu
    chunks = make_chunks(h)
    assert w == h, "chunking shared across axes"
    jpairs = _pairs(len(chunks))

    nc = bacc.Bacc("TRN2", target_bir_lowering=False, debug=False)
    xq_d = nc.dram_tensor("x_q", [n_imgs, h, w, c], u16, kind="ExternalInput")
    out_d = nc.dram_tensor("out", [n_imgs, h, w, c], u8, kind="ExternalOutput")
    # all chunk bands packed into one tensor pair: 412-byte-per-line
    # transfers run at ~2 GB/s (sub-packet descriptors); one 4KB-per-line
    # transfer streams at full rate
    nck = len(chunks)
    bandh_d = nc.dram_tensor("bandh", [128, 206 * nck], f16, kind="ExternalInput")
    bandl_d = nc.dram_tensor("bandl", [128, 206 * nck], f16, kind="ExternalInput")

    n_yblk = h // 128

    with tile.TileContext(nc) as tc:
        with (
            tc.tile_pool(name="const", bufs=1) as cpool,
            tc.tile_pool(name="xin", bufs=2) as xpool,
            tc.tile_pool(name="uv", bufs=1) as uvpool,
            tc.tile_pool(name="outp", bufs=2) as opool,
            tc.tile_pool(name="ps", bufs=3, space="PSUM") as pspool,
            tc.tile_pool(name="pso", bufs=3, space="PSUM") as psopool,
        ):
            # spread big DMAs over the 3 DMA-capable engine queues (a single
            # queue serializes and the 16-engine fabric is the bottleneck)
            dma_engines = [nc.sync, nc.scalar, nc.gpsimd]
            dma_rr = [0]

            def dma(dst, src):
                eng = dma_engines[dma_rr[0] % len(dma_engines)]
                dma_rr[0] += 1
                eng.dma_start(dst, src)

            # band constants in 3 per-chunk-pair piece tiles so each piece
            # can load just-in-time, interleaved with image-0's x chunks
            # (a monolithic band load ahead of the x loads starves the ramp)
            pieces = [(0, 412), (412, 824), (824, 206 * nck)]
            bandh_p, bandl_p, band32_p = [], [], []
            for k, (c0, c1) in enumerate(pieces):
                bh = cpool.tile([128, c1 - c0], f16, name=f"bandh_p{k}")
                bl = cpool.tile([128, c1 - c0], f16, name=f"bandl_p{k}")
                b32 = cpool.tile([128, c1 - c0], f32, name=f"band32_p{k}")
                bandh_p.append(bh)
                bandl_p.append(bl)
                band32_p.append(b32)

            def load_band_piece(k):
                c0, c1 = pieces[k]
                rows = [0, 43, 86, 128]
                for q in range(3):
                    a, bnd = rows[q], rows[q + 1]
                    dma_engines[q].dma_start(
                        bandh_p[k][a:bnd, :], bandh_d.ap()[a:bnd, c0:c1]
                    )
                    dma_engines[q].dma_start(
                        bandl_p[k][a:bnd, :], bandl_d.ap()[a:bnd, c0:c1]
                    )
                nc.vector.tensor_tensor(
                    band32_p[k][:], bandh_p[k][:], bandl_p[k][:],
                    mybir.AluOpType.add,
                )

            bandh, bandl, bandB, bandA = [], [], [], []
            for i, (s, e, lo, hi) in enumerate(chunks):
                rows, ni = hi - lo, e - s
                k, off = i // 2, (i % 2) * 206
                bandh.append(bandh_p[k][0:rows, off : off + 2 * ni])
                bandl.append(bandl_p[k][0:rows, off : off + 2 * ni])
                bandA.append(band32_p[k][0:rows, off : off + ni])
                bandB.append(band32_p[k][0:rows, off + ni : off + 2 * ni])

            # stage-1 psum drains: GpSimd cannot access PSUM; Vector takes
            # most (it cannot issue DMAs, so it has the most slack)
            drain_engines = [
                nc.vector.tensor_copy,
                nc.vector.tensor_copy,
                nc.scalar.copy,
                nc.vector.tensor_copy,
                nc.scalar.copy,
            ]

            for n in range(n_imgs):
                xhis, xlos = [], []
                for j, (s, e, lo, hi) in enumerate(chunks):
                    # just-in-time band piece loads, interleaved with image
                    # 0's chunks in queue order
                    if n == 0 and j % 2 == 0:
                        load_band_piece(j // 2)
                    xu = xpool.tile([hi - lo, w, c], u16, tag=f"xu{j}", name=f"xu{j}_{n}")
                    xh = xpool.tile([hi - lo, w, c], f16, tag=f"xh{j}", name=f"xh{j}_{n}")
                    xl = xpool.tile([hi - lo, w, c], f16, tag=f"xl{j}", name=f"xl{j}_{n}")
                    # split each chunk load over all 3 queues (during the
                    # 8-core startup burst one queue only sustains ~15 GB/s)
                    # and unpack each piece as it lands:
                    #   x_hi = f16(u16*s); x_lo = u16*s - x_hi (both Vector)
                    rows = hi - lo
                    # 32-aligned piece starts: vector ops require base
                    # partitions at multiples of 32. Also split by column
                    # halves at 256 px (= exactly 3 fabric packets per
                    # line): stage-1 work for the left x-chunks unlocks
                    # after only half a chunk arrives during the
                    # contention-limited ramp.
                    cuts = [0, 32, 64, rows]
                    for q in range(3):
                        a, bnd = cuts[q], cuts[q + 1]
                        for hh, (c0, c1) in enumerate(((0, 256), (256, 512))):
                            eng = dma_engines[(q * 2 + hh) % 3]
                            eng.dma_start(
                                xu[a:bnd, c0:c1, :],
                                xq_d.ap()[n, lo + a : lo + bnd, c0:c1, :],
                            )
                            nc.vector.tensor_scalar(
                                xh[a:bnd, c0:c1, :], xu[a:bnd, c0:c1, :],
                                float(QS), None, mybir.AluOpType.mult,
                            )
                            nc.vector.scalar_tensor_tensor(
                                xl[a:bnd, c0:c1, :], xu[a:bnd, c0:c1, :],
                                float(QS), xh[a:bnd, c0:c1, :],
                                mybir.AluOpType.mult, mybir.AluOpType.subtract,
                            )
                    xhis.append(xh)
                    xlos.append(xl)
                outs = []
                for b in range(n_yblk):
                    ot = opool.tile([128, w, c], u8, tag=f"o{b}", name=f"o{b}_{n}")
                    outs.append(ot)
                # uv tiles: plane 0 = u, plane 1 = v (transposed: x on
                # partitions); all 3 channels alive at once so the ramp
                # (image-0 chunks arriving at fabric rate) can feed the PE
                # with 3 channels' worth of stage-1 work per chunk pair
                uvts = [
                    [
                        uvpool.tile(
                            [hi - lo, 2, h], f32, tag=f"uv{ci}_{i}",
                            name=f"uv{ci}_{i}_{n}",
                        )
                        for i, (s, e, lo, hi) in enumerate(chunks)
                    ]
                    for ci in range(c)
                ]
                # stage 1: y-conv, transposed outputs; 3 fp16 matmuls per
                # (i,j); chunk-pair outer, channel inner
                for jp in jpairs:
                    nj = chunks[jp[0]][1] - chunks[jp[0]][0]
                    sj0 = chunks[jp[0]][0]
                    seg = 2 * nj
                    for ci in range(c):
                        for i, (si, ei, loi, hii) in enumerate(chunks):
                            mi = hii - loi
                            ps = pspool.tile([mi, 512], f32, tag="ps")
                            for t, j in enumerate(jp):
                                win = ps[:, t * seg : (t + 1) * seg]
                                hi_lhs = xhis[j][:, loi:hii, ci]
                                lo_lhs = xlos[j][:, loi:hii, ci]
                                nc.tensor.matmul(
                                    win, hi_lhs, bandh[j],
                                    start=True, stop=False,
                                )
                                nc.tensor.matmul(
                                    win, lo_lhs, bandh[j],
                                    start=False, stop=False,
                                )
                                nc.tensor.matmul(
                                    win, hi_lhs, bandl[j],
                                    start=False, stop=True,
                                )
                            # drain all segments with one 4D-AP copy
                            src = (
                                ps[:, 0 : len(jp) * seg]
                                .rearrange("m (js x) -> m js x", js=len(jp))
                                .rearrange("m js (uv x) -> m uv js x", uv=2)
                            )
                            dst = uvts[ci][i][
                                :, :, sj0 : sj0 + len(jp) * nj
                            ].rearrange("m uv (js x) -> m uv js x", js=len(jp))
                            drain_engines[i % 5](dst, src)
                # stage 2: x-conv; all 5 chunks accumulate into one PSUM
                # bank per 128-row block, single Relu(+1)->uint8 drain
                for ci in range(c):
                    for b in range(n_yblk):
                        pso = psopool.tile([128, 512], f32, tag="pso")
                        col = 0
                        for i, (s_, e_, lo_, hi_) in enumerate(chunks):
                            ni = e_ - s_
                            oslice = pso[:, col : col + ni]
                            nc.tensor.matmul(
                                oslice,
                                uvts[ci][i][:, 0, b * 128 : (b + 1) * 128],
                                bandB[i],
                                start=True,
                                stop=False,
                            )
                            nc.tensor.matmul(
                                oslice,
                                uvts[ci][i][:, 1, b * 128 : (b + 1) * 128],
                                bandA[i],
                                start=False,
                                stop=True,
                            )
                            col += ni
                        # Relu(psum + 1) -> uint8 saturates at 255: full clip
                        nc.scalar.activation(
                            outs[b][:, :, ci], pso[:, 0:col], relu, bias=1.0
                        )
                for b in range(n_yblk):
                    dma(out_d.ap()[n, b * 128 : (b + 1) * 128, :, :], outs[b][:])

    nc.compile()
    return nc


_CACHE = {}


def _get_nc():
    if "nc" not in _CACHE:
        _CACHE["nc"] = build_bass()
    return _CACHE["nc"]


def kernel(x: np.ndarray) -> np.ndarray:
    from concourse import bass_utils

    nc = _get_nc()
    bands64 = make_bands(H)
    nck = len(bands64)
    packh = np.zeros((128, 206 * nck), np.float16)
    packl = np.zeros((128, 206 * nck), np.float16)
    for i, b64 in enumerate(bands64):
        rows, seg = b64.shape
        bh = b64.astype(np.float16)
        bl = (b64 - bh.astype(np.float64)).astype(np.float16)
        packh[0:rows, 206 * i : 206 * i + seg] = bh
        packl[0:rows, 206 * i : 206 * i + seg] = bl
    const_map = {"bandh": packh, "bandl": packl}
    x_q = encode_x(x)
    in_maps = [
        {
            "x_q": x_q[k * IMG_PER_CORE : (k + 1) * IMG_PER_CORE],
            **const_map,
        }
        for k in range(N_CORES)
    ]
    res = bass_utils.run_bass_kernel_spmd(nc, in_maps, core_ids=list(range(N_CORES)))
    _CACHE["last_result"] = res
    out = np.concatenate([r["out"] for r in res.results], axis=0)
    return out.astype(np.float32)


# revision 37
# speedup vs baseline: 1.2325x; 1.1921x over previous
"""LoG (GaussianBlur 3x3 then Laplacian 9x9, reflect-101) as a Bass/Trainium kernel.

Math: both depthwise convs are separable and symmetric, and reflect-101
padding commutes with symmetric-kernel convolution, so the whole pipeline
composes into a single separable 11x11 filter applied to the reflect-101
extension of x:

    out = clip( (A (x) B + B (x) A) * x~  + 1, 0, 255 )

with A = gauss3 conv SMOOTH_9 (11 taps), B = gauss3 conv D2_9 (11 taps).

The run is DMA-fabric bound: the 16 device DMA engines sustain ~360 GB/s
shared by all 8 cores (~45 GB/s/core measured), so bytes moved dominate.
Input is therefore shipped as 2 bytes/elem: uint16 quantization with
noise-SHAPED dithering. The quantization error is spectrally shaped (33
feedback taps over the 3 previously-quantized rows, fully vectorized on
host) into the bands where the composed LoG filter has near-zero
response; measured end-to-end error of the u16 path is ~1.6 absolute
(vs 5.8 for naive u16 and a 5.1 budget incl. margin for other sources).
Output is uint8 (saturating engine cast). Bands ship as an fp16 hi/lo
pair; the fp32 stage-2 band is reconstructed on device (saves DMA).

Per core (batch 4 of the 32 images):
  unpack:  x_hi = f16(u16 * s) on Vector; x_lo = (u16 * s) - x_hi on
      GpSimd (scalar_tensor_tensor), both one pass per x-row chunk.
  stage 1 (y-conv, transposed output for free), fp16 matmuls (1 cyc/row
      vs fp32's 4): psum[x_chunk, 206] accumulates three matmuls
      x_hi^T@[A|B]_hi + x_lo^T@[A|B]_hi + x_hi^T@[A|B]_lo (band split
      hi/lo keeps the taps exact to 2^-21; dropped lo*lo is negligible).
      Two y-chunks share one PSUM bank, drained by a single 4D-AP copy
      into uv tiles.
  stage 2 (x-conv, back to original orientation), fp32 matmuls (u,v are
      ~6.5e4-scale; fp16 there would break the error budget): all 5
      x-chunks of one 128-row block accumulate into ONE PSUM bank (10
      matmuls), then a single Relu(psum + 1) activation per bank writes
      the uint8 output tile (saturates at 255, so the clip is free).

Big DMAs round-robin over the 3 DMA-capable queues (sync/scalar/gpsimd);
stage-1 drains go mostly to Vector. Reflect-101 boundaries are folded
into the per-chunk band matrices. x-chunks carry a +-5 y-halo so stage-2
lhsT reads stay partition-aligned.
"""

import numpy as np

N_CORES = 8
BATCH = 32
IMG_PER_CORE = BATCH // N_CORES
H = W = 512
C = 3
RAD = 5  # half width of composed 11-tap filter

QS = np.float32(255.0 / 65535.0)  # u16 quantization step


def make_chunks(n):
    step = 103  # 103*5 >= 512, in-size <= 113 <= 128
    bounds = list(range(0, n, step)) + [n]
    out = []
    for s, e in zip(bounds[:-1], bounds[1:]):
        lo, hi = max(s - RAD, 0), min(e + RAD, n)
        out.append((s, e, lo, hi))
    return out


CHUNKS = make_chunks(H)


def make_taps():
    g = np.exp(-((np.arange(3) - 1.0) ** 2) / 2.0)
    g = g / g.sum()
    S = np.array([1, 8, 28, 56, 70, 56, 28, 8, 1], dtype=np.float64)
    D2 = np.array([1, 4, 4, -4, -10, -4, 4, 4, 1], dtype=np.float64)
    return np.convolve(g, S), np.convolve(g, D2)


def make_bands(n):
    """Per chunk: [K, 2*Ni] = [BandA | BandB], reflect-101 folded in."""
    A, B = make_taps()
    bands = []
    for s, e, lo, hi in make_chunks(n):
        ni = e - s
        w = np.zeros((hi - lo, 2 * ni), np.float64)
        for j in range(ni):
            y = s + j
            for t in range(-RAD, RAD + 1):
                src = y + t
                if src < 0:
                    src = -src
                elif src > n - 1:
                    src = 2 * (n - 1) - src
                w[src - lo, j] += A[t + RAD]
                w[src - lo, ni + j] += B[t + RAD]
        bands.append(w)
    return bands


def _pairs(n):
    ps, i = [], 0
    while i < n:
        ps.append(tuple(range(i, min(i + 2, n))))
        i += 2
    return ps


def _shaping_taps():
    """LS-optimal noise-shaping feedback taps (3 previous rows, |dx|<=5),
    minimizing quantization-noise power through the composed LoG filter."""
    A, B = make_taps()
    lap2d = np.outer(A, B) + np.outer(B, A)
    N = 128
    f = np.fft.fftfreq(N) * 2 * np.pi
    FY, FX = np.meshgrid(f, f, indexing="ij")
    L = np.zeros((N, N), complex)
    for dy in range(11):
        for dx in range(11):
            L += lap2d[dy, dx] * np.exp(-1j * (FY * (dy - 5) + FX * (dx - 5)))
    P = np.abs(L) ** 2
    locs = [(dy, dx) for dy in range(1, 4) for dx in range(-5, 6)]
    E = [np.exp(-1j * (FY * dy + FX * dx)) for (dy, dx) in locs]
    n = len(E)
    M = np.zeros((n, n))
    b = np.zeros(n)
    for i in range(n):
        b[i] = -(np.real(E[i]) * P).mean()
        for j in range(i, n):
            M[i, j] = M[j, i] = (np.real(E[i] * np.conj(E[j])) * P).mean()
    t = np.linalg.solve(M, b)
    return locs, -t  # feedback taps: NTF = 1 - sum(fb z^-k) = 1 + sum(t z^-k)


_SHAPE_LOCS, _SHAPE_TAPS = _shaping_taps()


def encode_x(x):
    """Noise-shaped u16 quantization of x [N,H,W,C], vectorized over
    everything but the row index (error feedback over 3 previous rows)."""
    x = np.ascontiguousarray(x, dtype=np.float32)
    n, h, w, c = x.shape
    qs = np.float32(QS)
    err = np.zeros((4, n, w, c), np.float32)
    out = np.empty((n, h, w, c), np.uint16)
    for y in range(h):
        fb = np.zeros((n, w, c), np.float32)
        for (dy, dx), t in zip(_SHAPE_LOCS, _SHAPE_TAPS):
            e = err[dy]
            if dx == 0:
                fb += np.float32(t) * e
            elif dx > 0:
                fb[:, dx:, :] += np.float32(t) * e[:, :-dx, :]
            else:
                fb[:, :dx, :] += np.float32(t) * e[:, -dx:, :]
        target = x[:, y] - fb
        q = np.clip(np.rint(target / qs), 0, 65535)
        out[:, y] = q.astype(np.uint16)
        err = np.roll(err, 1, axis=0)
        err[1] = q.astype(np.float32) * qs - target
        err[0] = 0.0
    return out


def build_bass(n_imgs=IMG_PER_CORE, h=H, w=W, c=C):
    import concourse.bacc as bacc
    import concourse.mybir as mybir
    import concourse.tile as tile

    f32 = mybir.dt.float32
    f16 = mybir.dt.float16
    u8 = mybir.dt.uint8
    u16 = mybir.dt.uint16
    relu = mybir.ActivationFunctionType.Relu
    chunks = make_chunks(h)
    assert w == h, "chunking shared across axes"
    jpairs = _pairs(len(chunks))

    nc = bacc.Bacc("TRN2", target_bir_lowering=False, debug=False)
    xq_d = nc.dram_tensor("x_q", [n_imgs, h, w, c], u16, kind="ExternalInput")
    out_d = nc.dram_tensor("out", [n_imgs, h, w, c], u8, kind="ExternalOutput")
    # all chunk bands packed into one tensor pair: 412-byte-per-line
    # transfers run at ~2 GB/s (sub-packet descriptors); one 4KB-per-line
    # transfer streams at full rate
    nck = len(chunks)
    bandh_d = nc.dram_tensor("bandh", [128, 206 * nck], f16, kind="ExternalInput")
    bandl_d = nc.dram_tensor("bandl", [128, 206 * nck], f16, kind="ExternalInput")

    n_yblk = h // 128

    with tile.TileContext(nc) as tc:
        with (
            tc.tile_pool(name="const", bufs=1) as cpool,
            tc.tile_pool(name="xin", bufs=2) as xpool,
            tc.tile_pool(name="uv", bufs=1) as uvpool,
            tc.tile_pool(name="outp", bufs=2) as opool,
            tc.tile_pool(name="ps", bufs=4, space="PSUM") as pspool,
            tc.tile_pool(name="pso", bufs=3, space="PSUM") as psopool,
        ):
            # spread big DMAs over the 3 DMA-capable engine queues (a single
            # queue serializes and the 16-engine fabric is the bottleneck)
            dma_engines = [nc.sync, nc.scalar, nc.gpsimd]
            dma_rr = [0]

            def dma(dst, src):
                eng = dma_engines[dma_rr[0] % len(dma_engines)]
                dma_rr[0] += 1
                eng.dma_start(dst, src)

            # band constants in 3 per-chunk-pair piece tiles so each piece
            # can load just-in-time, interleaved with image-0's x chunks
            # (a monolithic band load ahead of the x loads starves the ramp)
            pieces = [(0, 412), (412, 824), (824, 206 * nck)]
            bandh_p, bandl_p, band32_p = [], [], []
            for k, (c0, c1) in enumerate(pieces):
                bh = cpool.tile([128, c1 - c0], f16, name=f"bandh_p{k}")
                bl = cpool.tile([128, c1 - c0], f16, name=f"bandl_p{k}")
                b32 = cpool.tile([128, c1 - c0], f32, name=f"band32_p{k}")
                bandh_p.append(bh)
                bandl_p.append(bl)
                band32_p.append(b32)

            def load_band_piece(k):
                c0, c1 = pieces[k]
                rows = [0, 43, 86, 128]
                for q in range(3):
                    a, bnd = rows[q], rows[q + 1]
                    dma_engines[q].dma_start(
                        bandh_p[k][a:bnd, :], bandh_d.ap()[a:bnd, c0:c1]
                    )
                    dma_engines[q].dma_start(
                        bandl_p[k][a:bnd, :], bandl_d.ap()[a:bnd, c0:c1]
                    )
                nc.vector.tensor_tensor(
                    band32_p[k][:], bandh_p[k][:], bandl_p[k][:],
                    mybir.AluOpType.add,
                )

            bandh, bandl, bandB, bandA = [], [], [], []
            for i, (s, e, lo, hi) in enumerate(chunks):
                rows, ni = hi - lo, e - s
                k, off = i // 2, (i % 2) * 206
                bandh.append(bandh_p[k][0:rows, off : off + 2 * ni])
                bandl.append(bandl_p[k][0:rows, off : off + 2 * ni])
                bandA.append(band32_p[k][0:rows, off : off + ni])
                bandB.append(band32_p[k][0:rows, off + ni : off + 2 * ni])

            # stage-1 psum drains: GpSimd cannot access PSUM; Vector takes
            # most (it cannot issue DMAs, so it has the most slack)
            drain_engines = [
                nc.vector.tensor_copy,
                nc.vector.tensor_copy,
                nc.scalar.copy,
                nc.vector.tensor_copy,
                nc.scalar.copy,
            ]

            for n in range(n_imgs):
                xhis, xlos = [], []
                for j, (s, e, lo, hi) in enumerate(chunks):
                    # just-in-time band piece loads, interleaved with image
                    # 0's chunks in queue order
                    if n == 0 and j % 2 == 0:
                        load_band_piece(j // 2)
                    xu = xpool.tile([hi - lo, w, c], u16, tag=f"xu{j}", name=f"xu{j}_{n}")
                    xh = xpool.tile([hi - lo, w, c], f16, tag=f"xh{j}", name=f"xh{j}_{n}")
                    xl = xpool.tile([hi - lo, w, c], f16, tag=f"xl{j}", name=f"xl{j}_{n}")
                    # split each chunk load over all 3 queues (during the
                    # 8-core startup burst one queue only sustains ~15 GB/s)
                    # and unpack each piece as it lands:
                    #   x_hi = f16(u16*s); x_lo = u16*s - x_hi (both Vector)
                    rows = hi - lo
                    # 32-aligned piece starts: vector ops require base
                    # partitions at multiples of 32
                    cuts = [0, 32, 64, rows]
                    for q in range(3):
                        a, bnd = cuts[q], cuts[q + 1]
                        dma_engines[q].dma_start(
                            xu[a:bnd, :, :], xq_d.ap()[n, lo + a : lo + bnd, :, :]
                        )
                        nc.vector.tensor_scalar(
                            xh[a:bnd, :, :], xu[a:bnd, :, :],
                            float(QS), None, mybir.AluOpType.mult,
                        )
                        nc.vector.scalar_tensor_tensor(
                            xl[a:bnd, :, :], xu[a:bnd, :, :], float(QS),
                            xh[a:bnd, :, :],
                            mybir.AluOpType.mult, mybir.AluOpType.subtract,
                        )
                    xhis.append(xh)
                    xlos.append(xl)
                outs = []
                for b in range(n_yblk):
                    ot = opool.tile([128, w, c], u8, tag=f"o{b}", name=f"o{b}_{n}")
                    outs.append(ot)
                # uv tiles: plane 0 = u, plane 1 = v (transposed: x on
                # partitions); all 3 channels alive at once so the ramp
                # (image-0 chunks arriving at fabric rate) can feed the PE
                # with 3 channels' worth of stage-1 work per chunk pair
                uvts = [
                    [
                        uvpool.tile(
                            [hi - lo, 2, h], f32, tag=f"uv{ci}_{i}",
                            name=f"uv{ci}_{i}_{n}",
                        )
                        for i, (s, e, lo, hi) in enumerate(chunks)
                    ]
                    for ci in range(c)
                ]
                # stage 1: y-conv, transposed outputs; 3 fp16 matmuls per
                # (i,j); chunk-pair outer, channel inner
                for jp in jpairs:
                    nj = chunks[jp[0]][1] - chunks[jp[0]][0]
                    sj0 = chunks[jp[0]][0]
                    seg = 2 * nj
                    for ci in range(c):
                        for i, (si, ei, loi, hii) in enumerate(chunks):
                            mi = hii - loi
                            ps = pspool.tile([mi, 512], f32, tag="ps")
                            for t, j in enumerate(jp):
                                win = ps[:, t * seg : (t + 1) * seg]
                                hi_lhs = xhis[j][:, loi:hii, ci]
                                lo_lhs = xlos[j][:, loi:hii, ci]
                                nc.tensor.matmul(
                                    win, hi_lhs, bandh[j],
                                    start=True, stop=False,
                                )
                                nc.tensor.matmul(
                                    win, lo_lhs, bandh[j],
                                    start=False, stop=False,
                                )
                                nc.tensor.matmul(
                                    win, hi_lhs, bandl[j],
                                    start=False, stop=True,
                                )
                            # drain all segments with one 4D-AP copy
                            src = (
                                ps[:, 0 : len(jp) * seg]
                                .rearrange("m (js x) -> m js x", js=len(jp))
                                .rearrange("m js (uv x) -> m uv js x", uv=2)
                            )
                            dst = uvts[ci][i][
                                :, :, sj0 : sj0 + len(jp) * nj
                            ].rearrange("m uv (js x) -> m uv js x", js=len(jp))
                            drain_engines[i % 5](dst, src)
                # stage 2: x-conv; all 5 chunks accumulate into one PSUM
                # bank per 128-row block, single Relu(+1)->uint8 drain
                for ci in range(c):
                    for b in range(n_yblk):
                        pso = psopool.tile([128, 512], f32, tag="pso")
                        col = 0
                        for i, (s_, e_, lo_, hi_) in enumerate(chunks):
                            ni = e_ - s_
                            oslice = pso[:, col : col + ni]
                            nc.tensor.matmul(
                                oslice,
                                uvts[ci][i][:, 0, b * 128 : (b + 1) * 128],
                                bandB[i],
                                start=True,
                                stop=False,
                            )
                            nc.tensor.matmul(
                                oslice,
                                uvts[ci][i][:, 1, b * 128 : (b + 1) * 128],
                                bandA[i],
                                start=False,
                                stop=True,
                            )
                            col += ni
                        # Relu(psum + 1) -> uint8 saturates at 255: full clip
                        nc.scalar.activation(
                            outs[b][:, :, ci], pso[:, 0:col], relu, bias=1.0
                        )
                for b in range(n_yblk):
                    dma(out_d.ap()[n, b * 128 : (b + 1) * 128, :, :], outs[b][:])

    nc.compile()
    return nc


_CACHE = {}


def _get_nc():
    if "nc" not in _CACHE:
        _CACHE["nc"] = build_bass()
    return _CACHE["nc"]


def kernel(x: np.ndarray) -> np.ndarray:
    from concourse import bass_utils

    nc = _get_nc()
    bands64 = make_bands(H)
    nck = len(bands64)
    packh = np.zeros((128, 206 * nck), np.float16)
    packl = np.zeros((128, 206 * nck), np.float16)
    for i, b64 in enumerate(bands64):
        rows, seg = b64.shape
        bh = b64.astype(np.float16)
        bl = (b64 - bh.astype(np.float64)).astype(np.float16)
        packh[0:rows, 206 * i : 206 * i + seg] = bh
        packl[0:rows, 206 * i : 206 * i + seg] = bl
    const_map = {"bandh": packh, "bandl": packl}
    x_q = encode_x(x)
    in_maps = [
        {
            "x_q": x_q[k * IMG_PER_CORE : (k + 1) * IMG_PER_CORE],
            **const_map,
        }
        for k in range(N_CORES)
    ]
    res = bass_utils.run_bass_kernel_spmd(nc, in_maps, core_ids=list(range(N_CORES)))
    _CACHE["last_result"] = res
    out = np.concatenate([r["out"] for r in res.results], axis=0)
    return out.astype(np.float32)
